# revision 10
# baseline (speedup 1.0000x reference)
"""Trainium2 Bass kernel for nn_ConvolutionalAttention_3015067042131.

Math (reference.py):
  x [16,128,64,64] f32; x1 = x[:, :64], x2 = x[:, 64:]
  pooled = mean(x1, HW); h = gelu(pooled @ w1.T + b1); dyn = (h @ w2.T + b2) -> [B,64,9]
  x1_dyn = per-(batch,channel) 3x3 depthwise conv of x1 with dyn
  x1_lk  = conv2d(x1, lk_filter[64,64,13,13], SAME)
  out = concat([x1_lk + x1_dyn, x2], ch)

Strategy:
  * The tiny MLP (dyn) is computed on host in float64 (0.0007% of FLOPs).
  * The dynamic depthwise 3x3 is folded into the 13x13 conv weights as
    per-batch diagonal additions on the central 3x3 taps (3x3 tap (u,v)
    == 13x13 tap (u+5, v+5)).
  * Conv as shift-and-matmul: for each kernel tap, out[o, pix] +=
    W_tap[c, o].T @ xpad[c, pix+off]. Taps are paired along K=128:
      - xp layout: partitions 0-63 hold the zero-padded 76x76 image,
        64-127 hold it shifted LEFT one column -> taps (i,2j),(i,2j+1)
        fuse into one matmul. 78 tiles cover columns 0-11.
      - xq layout: partitions 64-127 hold the image shifted UP one row
        -> taps (2i,12),(2i+1,12) fuse. 6 tiles cover column 12 rows
        0-11, plus 1 single tile for tap (12,12).
    85 tiles total for 169 taps (optimal: 84 pairs + 1 single).
  * Loop order: batch outer, tap-tile outer, chunk inner. Each batch's
    full output (4 chunk-pairs x [128,512] f32) stays RESIDENT in PSUM
    (8 banks = exactly 2 images), accumulating across all 85 taps; one
    drain at the end of each batch, pipelined per bank.
  * Chunk pairs run CONCURRENTLY in the two PE column halves via
    tile_position (0,0)/(0,64) writing PSUM partitions 0-63/64-127.
  * fp16 operands (HW fp16 matmul, fp32 PSUM accumulate; end-to-end rel
    err ~3e-4). f32r is broken in this stack; fp32 runs at 1/4 rate;
    fp8 measured 3.8e-2 rel err on this data -> over the 2e-2 gate.
  * Head: the framework preamble is ~7.2us; immediately after it, junk
    matmuls ramp the PE clock (1.2->2.4GHz takes ~3us of sustained PE
    activity) while image DMAs (sync queue) + weight DMAs (scalar
    queue) land and DVE builds the padded layouts. Border memsets run
    during the DMA wait; per-batch layouts build one batch ahead.
  * Sharding: data-parallel over batch, 2 batches per core on 8 cores.
    x2 passthrough is host-side (no device work).
"""
import math

import numpy as np

B, C, H, W = 16, 128, 64, 64
PDIM, SK, LK = 64, 3, 13
PAD = LK // 2  # 6
HP, WP = H + 2 * PAD, W + 2 * PAD  # 76, 76
NCORES = 8
BPC = B // NCORES  # batches per core
NT = 85            # weight tiles (84 tap pairs + 1 single)
NCHUNK = 8         # 512-pixel chunks per image
CHUNK = H * W // NCHUNK  # 512
NWARM = 14         # clock-ramp junk matmuls (N=256 each)

# tile t: t < 78 -> col-pair, A tap (t//6, 2*(t%6)), B = (i, j+1), layout xp
#         78 <= t < 84 -> row-pair, A tap (2*(t-78), 12), B = (i+1, 12), xq
#         t == 84 -> single tap (12, 12), B-half weights zero, layout xp


def _tile_tap(t):
    if t < 78:
        return t // 6, 2 * (t % 6)
    if t < 84:
        return 2 * (t - 78), 12
    return 12, 12


# central 3x3 taps (i,j in 5..7): j=5 -> tile i*6+2 B-half; j=6/7 -> tile
# i*6+3 A/B-half; those 6 tiles are per-batch.
_MOD_TILES = [5 * 6 + 2, 6 * 6 + 2, 7 * 6 + 2, 5 * 6 + 3, 6 * 6 + 3, 7 * 6 + 3]
_MOD_SLOT = {t: s for s, t in enumerate(_MOD_TILES)}

_ERF = np.vectorize(math.erf, otypes=[np.float64])

_CACHED_NC = None


def _build_nc():
    import concourse.mybir as mybir
    import concourse.tile as tile
    from concourse import bacc

    f32 = mybir.dt.float32
    f16 = mybir.dt.float16

    nc = bacc.Bacc(None, target_bir_lowering=False)
    # host-pre-padded image, flat 77x76 rows per channel: rows 0-75 = the
    # zero-padded 76x76 image, row 76 = zeros. The three on-chip layouts
    # are then pure contiguous DMAs at flat offsets 0 / +1 (left-shift
    # wraps col 75 onto the next row's zero pad col) / +76 (up-shift
    # wraps row 75 onto the zero row 76).
    xsp = nc.dram_tensor("xsp", [BPC, PDIM, (HP + 1) * WP], f16, kind="ExternalInput")
    wsh = nc.dram_tensor("wsh", [128, NT * 64], f16, kind="ExternalInput")
    wmod = nc.dram_tensor("wmod", [BPC, 128, 6 * 64], f16, kind="ExternalInput")
    # chunk-major output so each PSUM bank drains in ONE dma
    y = nc.dram_tensor("y", [BPC, NCHUNK, PDIM, CHUNK], f32, kind="ExternalOutput")

    with tile.TileContext(nc) as tc:
        with (
            tc.tile_pool(name="wpool", bufs=1) as wpool,
            tc.tile_pool(name="wmpool", bufs=1) as wmpool,
            tc.tile_pool(name="xppool", bufs=1) as xppool,
            tc.tile_pool(name="xqpool", bufs=1) as xqpool,
            tc.tile_pool(name="opool", bufs=1) as opool,
            tc.tile_pool(name="pspool", bufs=1, space="PSUM") as pspool,
        ):
            # 8 PSUM banks = exactly the per-core output (2 images x 4
            # chunk-pair banks); resident for the whole batch accumulation
            ps_t = [
                [
                    pspool.tile([128, CHUNK], f32, name=f"ps{b}{cp}")
                    for cp in range(4)
                ]
                for b in range(BPC)
            ]
            # PE warmup: junk matmuls on a zeroed scratch tile so the HAM
            # un-throttles (1.2 -> 2.4 GHz) while input DMAs + layout
            # build run. N=256 keeps each one cheap. Targets the LAST bank
            # to be opened for real accumulation (b1 cp3, ~77us away).
            scratch = wpool.tile([128, 512], f16)
            nc.vector.memset(scratch[:], 0.0)
            for wi in range(NWARM):
                nc.tensor.matmul(
                    ps_t[BPC - 1][3][0:64, 0:256],
                    lhsT=scratch[:, 0:64],
                    rhs=scratch[:, 0:256],
                    start=(wi == 0),
                    stop=(wi == NWARM - 1),
                    skip_group_check=True,
                )

            # weights on the scalar DMA queue (sync queue carries images);
            # first 12 tiles separately so tap 0 unblocks early
            wsh_sb = wpool.tile([128, NT * 64], f16)
            nc.scalar.dma_start(out=wsh_sb[:, 0 : 12 * 64], in_=wsh[:, 0 : 12 * 64])
            nc.scalar.dma_start(out=wsh_sb[:, 12 * 64 :], in_=wsh[:, 12 * 64 :])

            NPIX = HP * WP  # 5776
            xp_t, xq_t, wm_t = [], [], []
            for b in range(BPC):
                wm = wmpool.tile([128, 6 * 64], f16, name=f"wm{b}")
                nc.scalar.dma_start(out=wm[:], in_=wmod[b, :, :])
                # partitions 0-63: padded image; 64-127: shifted left 1 col
                # (xp) / shifted up 1 row (xq) — all contiguous flat DMAs
                xp = xppool.tile([128, HP, WP], f16, name=f"xp{b}")
                xq = xqpool.tile([128, HP, WP], f16, name=f"xq{b}")
                nc.sync.dma_start(out=xp[0:64, :, :], in_=xsp[b, :, 0:NPIX])
                nc.sync.dma_start(out=xp[64:128, :, :], in_=xsp[b, :, 1 : NPIX + 1])
                nc.sync.dma_start(out=xq[0:64, :, :], in_=xsp[b, :, 0:NPIX])
                nc.sync.dma_start(out=xq[64:128, :, :], in_=xsp[b, :, WP : NPIX + WP])
                xp_t.append(xp)
                xq_t.append(xq)
                wm_t.append(wm)

            for b in range(BPC):
                for t in range(NT):
                    s = _MOD_SLOT.get(t)
                    w_ap = (
                        wm_t[b][:, s * 64 : (s + 1) * 64]
                        if s is not None
                        else wsh_sb[:, t * 64 : (t + 1) * 64]
                    )
                    i, j = _tile_tap(t)
                    xt = xq_t[b] if 78 <= t < 84 else xp_t[b]
                    for cp in range(4):
                        for half in (0, 1):
                            r0 = i + 8 * (2 * cp + half)
                            nc.tensor.matmul(
                                ps_t[b][cp][64 * half : 64 * (half + 1), :],
                                lhsT=w_ap,
                                rhs=xt[:, r0 : r0 + 8, j : j + 64],
                                start=(t == 0),
                                stop=(t == NT - 1),
                                tile_position=(0, 64 * half),
                                skip_group_check=True,
                            )
                for cp in range(4):
                    ot = opool.tile([128, CHUNK], f32, name=f"ot{b}{cp}")
                    nc.vector.tensor_copy(ot[:], ps_t[b][cp][:])
                    # chunk-major y: [2 chunks, 64 ch, 512 px] flat-matches
                    # ot's [128, 512]; one DMA per bank, triggers split
                    # across the two hwdge queues to shorten the tail
                    eng = nc.sync if cp < 2 else nc.scalar
                    eng.dma_start(
                        out=y[b, 2 * cp : 2 * cp + 2, :, :], in_=ot[:, :]
                    )
    nc.compile()
    return nc


def _get_nc():
    global _CACHED_NC
    if _CACHED_NC is None:
        _CACHED_NC = _build_nc()
    return _CACHED_NC


def _host_dyn(x, w1, b1, w2, b2):
    """dwc_proj MLP on host, float64: dyn [B, 64, 9]."""
    pooled = x[:, :PDIM].mean(axis=(2, 3), dtype=np.float64)      # [B, 64]
    z = pooled @ w1.T.astype(np.float64) + b1.astype(np.float64)  # [B, 32]
    h = 0.5 * z * (1.0 + _ERF(z / math.sqrt(2.0)))                # exact gelu
    dyn = h @ w2.T.astype(np.float64) + b2.astype(np.float64)     # [B, 576]
    return dyn.reshape(B, PDIM, SK * SK)


def _host_weights(lk_filter, dyn):
    """Build shared tap-pair weight tiles + per-batch modified central tiles.

    Weight tile t [128, 64]: rows 0-63 = lk[o, c, iA, jA].T (tap A), rows
    64-127 = tap B, zeros for the single. lhsT layout [K=c, M=o].
    """
    lkT = lk_filter.transpose(1, 0, 2, 3).astype(np.float32)  # [c, o, i, j]
    Wt = np.zeros((NT, 128, 64), np.float32)
    for t in range(NT):
        i, jA = _tile_tap(t)
        Wt[t, 0:64, :] = lkT[:, :, i, jA]
        if t < 78:
            Wt[t, 64:128, :] = lkT[:, :, i, jA + 1]
        elif t < 84:
            Wt[t, 64:128, :] = lkT[:, :, i + 1, jA]

    ar = np.arange(64)
    Wmod = np.zeros((B, 6, 128, 64), np.float32)
    for ii, i in enumerate((5, 6, 7)):
        t2, t3 = i * 6 + 2, i * 6 + 3
        u = i - 5
        for b in range(B):
            m2 = Wt[t2].copy()
            m3 = Wt[t3].copy()
            m2[64 + ar, ar] += dyn[b, :, u * 3 + 0].astype(np.float32)  # tap (i,5)
            m3[ar, ar] += dyn[b, :, u * 3 + 1].astype(np.float32)       # tap (i,6)
            m3[64 + ar, ar] += dyn[b, :, u * 3 + 2].astype(np.float32)  # tap (i,7)
            Wmod[b, ii] = m2
            Wmod[b, 3 + ii] = m3

    wsh_np = np.ascontiguousarray(
        Wt.transpose(1, 0, 2).reshape(128, NT * 64)
    ).astype(np.float16)
    wmod_np = np.ascontiguousarray(
        Wmod.transpose(0, 2, 1, 3).reshape(B, 128, 6 * 64)
    ).astype(np.float16)
    return wsh_np, wmod_np


def _make_in_maps(x, lk_filter, w1, b1, w2, b2):
    x = np.asarray(x, dtype=np.float32)
    dyn = _host_dyn(x, np.asarray(w1), np.asarray(b1), np.asarray(w2), np.asarray(b2))
    wsh_np, wmod_np = _host_weights(np.asarray(lk_filter, dtype=np.float32), dyn)

    # flat 77x76-row padded image per channel (see _build_nc)
    xsp_np = np.zeros((B, PDIM, HP + 1, WP), np.float16)
    xsp_np[:, :, PAD : PAD + H, PAD : PAD + W] = x[:, :PDIM]
    xsp_np = xsp_np.reshape(B, PDIM, (HP + 1) * WP)

    in_maps = []
    for k in range(NCORES):
        b0 = k * BPC
        in_maps.append(
            {
                "xsp": np.ascontiguousarray(xsp_np[b0 : b0 + BPC]),
                "wsh": wsh_np,
                "wmod": np.ascontiguousarray(wmod_np[b0 : b0 + BPC]),
            }
        )
    return in_maps


def kernel(x, lk_filter, w1, b1, w2, b2):
    from concourse.bass_utils import run_bass_kernel_spmd

    x = np.asarray(x, dtype=np.float32)
    in_maps = _make_in_maps(x, lk_filter, w1, b1, w2, b2)
    nc = _get_nc()
    res = run_bass_kernel_spmd(nc, in_maps, core_ids=list(range(NCORES)))

    out = np.empty((B, C, H, W), np.float32)
    for k in range(NCORES):
        b0 = k * BPC
        # y is chunk-major [BPC, 8, 64, 512] -> [BPC, 64, 64, 64]
        yk = res.results[k]["y"].reshape(BPC, NCHUNK, PDIM, 8, W)
        out[b0 : b0 + BPC, :PDIM] = yk.transpose(0, 2, 1, 3, 4).reshape(
            BPC, PDIM, H, W
        )
    out[:, PDIM:] = x[:, PDIM:]
    return out


# revision 12
# speedup vs baseline: 1.0166x; 1.0166x over previous
"""Trainium2 Bass kernel for nn_ConvolutionalAttention_3015067042131.

Math (reference.py):
  x [16,128,64,64] f32; x1 = x[:, :64], x2 = x[:, 64:]
  pooled = mean(x1, HW); h = gelu(pooled @ w1.T + b1); dyn = (h @ w2.T + b2) -> [B,64,9]
  x1_dyn = per-(batch,channel) 3x3 depthwise conv of x1 with dyn
  x1_lk  = conv2d(x1, lk_filter[64,64,13,13], SAME)
  out = concat([x1_lk + x1_dyn, x2], ch)

Strategy:
  * The tiny MLP (dyn) is computed on host in float64 (0.0007% of FLOPs).
  * The dynamic depthwise 3x3 is folded into the 13x13 conv weights as
    per-batch diagonal additions on the central 3x3 taps (3x3 tap (u,v)
    == 13x13 tap (u+5, v+5)).
  * Conv as shift-and-matmul: for each kernel tap, out[o, pix] +=
    W_tap[c, o].T @ xpad[c, pix+off]. Taps are paired along K=128:
      - xp layout: partitions 0-63 hold the zero-padded 76x76 image,
        64-127 hold it shifted LEFT one column -> taps (i,2j),(i,2j+1)
        fuse into one matmul. 78 tiles cover columns 0-11.
      - xq layout: partitions 64-127 hold the image shifted UP one row
        -> taps (2i,12),(2i+1,12) fuse. 6 tiles cover column 12 rows
        0-11, plus 1 single tile for tap (12,12).
    85 tiles total for 169 taps (optimal: 84 pairs + 1 single).
  * Loop order: batch outer, tap-tile outer, chunk inner. Each batch's
    full output (4 chunk-pairs x [128,512] f32) stays RESIDENT in PSUM
    (8 banks = exactly 2 images), accumulating across all 85 taps; one
    drain at the end of each batch, pipelined per bank.
  * Chunk pairs run CONCURRENTLY in the two PE column halves via
    tile_position (0,0)/(0,64) writing PSUM partitions 0-63/64-127.
  * fp16 operands (HW fp16 matmul, fp32 PSUM accumulate; end-to-end rel
    err ~3e-4). f32r is broken in this stack; fp32 runs at 1/4 rate;
    fp8 measured 3.8e-2 rel err on this data -> over the 2e-2 gate.
  * Head: the framework preamble is ~7.2us; immediately after it, junk
    matmuls ramp the PE clock (1.2->2.4GHz takes ~3us of sustained PE
    activity) while image DMAs (sync queue) + weight DMAs (scalar
    queue) land and DVE builds the padded layouts. Border memsets run
    during the DMA wait; per-batch layouts build one batch ahead.
  * Sharding: data-parallel over batch, 2 batches per core on 8 cores.
    x2 passthrough is host-side (no device work).
"""
import math

import numpy as np

B, C, H, W = 16, 128, 64, 64
PDIM, SK, LK = 64, 3, 13
PAD = LK // 2  # 6
HP, WP = H + 2 * PAD, W + 2 * PAD  # 76, 76
NCORES = 8
BPC = B // NCORES  # batches per core
NT = 85            # weight tiles (84 tap pairs + 1 single)
NCHUNK = 8         # 512-pixel chunks per image
CHUNK = H * W // NCHUNK  # 512
NWARM = 16         # clock-ramp junk matmuls (N=256 each)

# tile t: t < 78 -> col-pair, A tap (t//6, 2*(t%6)), B = (i, j+1), layout xp
#         78 <= t < 84 -> row-pair, A tap (2*(t-78), 12), B = (i+1, 12), xq
#         t == 84 -> single tap (12, 12), B-half weights zero, layout xp


def _tile_tap(t):
    if t < 78:
        return t // 6, 2 * (t % 6)
    if t < 84:
        return 2 * (t - 78), 12
    return 12, 12


# central 3x3 taps (i,j in 5..7): j=5 -> tile i*6+2 B-half; j=6/7 -> tile
# i*6+3 A/B-half; those 6 tiles are per-batch.
_MOD_TILES = [5 * 6 + 2, 6 * 6 + 2, 7 * 6 + 2, 5 * 6 + 3, 6 * 6 + 3, 7 * 6 + 3]
_MOD_SLOT = {t: s for s, t in enumerate(_MOD_TILES)}

_ERF = np.vectorize(math.erf, otypes=[np.float64])

_CACHED_NC = None


def _build_nc():
    import concourse.mybir as mybir
    import concourse.tile as tile
    from concourse import bacc

    f32 = mybir.dt.float32
    f16 = mybir.dt.float16

    nc = bacc.Bacc(None, target_bir_lowering=False)
    # host-pre-padded image, flat 77x76 rows per channel: rows 0-75 = the
    # zero-padded 76x76 image, row 76 = zeros. The three on-chip layouts
    # are then pure contiguous DMAs at flat offsets 0 / +1 (left-shift
    # wraps col 75 onto the next row's zero pad col) / +76 (up-shift
    # wraps row 75 onto the zero row 76).
    xsp = nc.dram_tensor("xsp", [BPC, PDIM, (HP + 1) * WP], f16, kind="ExternalInput")
    wsh = nc.dram_tensor("wsh", [128, NT * 64], f16, kind="ExternalInput")
    wmod = nc.dram_tensor("wmod", [BPC, 128, 6 * 64], f16, kind="ExternalInput")
    # chunk-major output so each PSUM bank drains in ONE dma
    y = nc.dram_tensor("y", [BPC, NCHUNK, PDIM, CHUNK], f32, kind="ExternalOutput")

    with tile.TileContext(nc) as tc:
        with (
            tc.tile_pool(name="wpool", bufs=1) as wpool,
            tc.tile_pool(name="wmpool", bufs=1) as wmpool,
            tc.tile_pool(name="xppool", bufs=1) as xppool,
            tc.tile_pool(name="xqpool", bufs=1) as xqpool,
            tc.tile_pool(name="opool", bufs=1) as opool,
            tc.tile_pool(name="pspool", bufs=1, space="PSUM") as pspool,
        ):
            # 8 PSUM banks = exactly the per-core output (2 images x 4
            # chunk-pair banks); resident for the whole batch accumulation
            ps_t = [
                [
                    pspool.tile([128, CHUNK], f32, name=f"ps{b}{cp}")
                    for cp in range(4)
                ]
                for b in range(BPC)
            ]
            # PE warmup: junk matmuls on a zeroed scratch tile so the HAM
            # un-throttles (1.2 -> 2.4 GHz) while input DMAs + layout
            # build run. N=256 keeps each one cheap. Targets the LAST bank
            # to be opened for real accumulation (b1 cp3, ~77us away).
            scratch = wpool.tile([128, 512], f16)
            nc.vector.memset(scratch[:], 0.0)
            for wi in range(NWARM):
                nc.tensor.matmul(
                    ps_t[BPC - 1][3][0:64, 0:256],
                    lhsT=scratch[:, 0:64],
                    rhs=scratch[:, 0:256],
                    start=(wi == 0),
                    stop=(wi == NWARM - 1),
                    skip_group_check=True,
                )

            # The DMA queue is determined by the trigger engine (sync -> q1,
            # scalar -> q10, ~170GB/s each), so the batch-0 critical bytes
            # are split across BOTH queues, row-split so region deps let
            # early chunk-pairs start before the full image lands.
            NPIX = HP * WP  # 5776
            RS = 48 * WP    # row-split point (rows 0-47 / 48-75)
            wsh_sb = wpool.tile([128, NT * 64], f16)
            xp_t = [xppool.tile([128, HP, WP], f16, name=f"xp{b}") for b in range(BPC)]
            xq_t = [xqpool.tile([128, HP, WP], f16, name=f"xq{b}") for b in range(BPC)]
            wm_t = [wmpool.tile([128, 6 * 64], f16, name=f"wm{b}") for b in range(BPC)]

            # sync queue: xp0 A-half pieces, then mid weight chunk
            nc.sync.dma_start(out=xp_t[0][0:64, 0:48, :], in_=xsp[0, :, 0:RS])
            nc.sync.dma_start(out=xp_t[0][0:64, 48:, :], in_=xsp[0, :, RS:NPIX])
            nc.sync.dma_start(
                out=wsh_sb[:, 8 * 64 : 31 * 64], in_=wsh[:, 8 * 64 : 31 * 64]
            )
            # scalar queue: first weight tiles (tiny), xp0 B-half pieces,
            # then the weight tail
            nc.scalar.dma_start(out=wsh_sb[:, 0 : 8 * 64], in_=wsh[:, 0 : 8 * 64])
            nc.scalar.dma_start(out=xp_t[0][64:128, 0:48, :], in_=xsp[0, :, 1 : RS + 1])
            nc.scalar.dma_start(
                out=xp_t[0][64:128, 48:, :], in_=xsp[0, :, RS + 1 : NPIX + 1]
            )
            nc.scalar.dma_start(out=wsh_sb[:, 31 * 64 :], in_=wsh[:, 31 * 64 :])
            # non-critical inputs (xq0 needed ~60us in, batch 1 ~80us in);
            # must still be issued before any drain trigger shares a queue
            nc.scalar.dma_start(out=wm_t[0][:], in_=wmod[0, :, :])
            nc.sync.dma_start(out=xq_t[0][0:64, :, :], in_=xsp[0, :, 0:NPIX])
            nc.scalar.dma_start(out=xq_t[0][64:128, :, :], in_=xsp[0, :, WP : NPIX + WP])
            nc.sync.dma_start(out=xp_t[1][0:64, :, :], in_=xsp[1, :, 0:NPIX])
            nc.scalar.dma_start(out=xp_t[1][64:128, :, :], in_=xsp[1, :, 1 : NPIX + 1])
            nc.sync.dma_start(out=xq_t[1][0:64, :, :], in_=xsp[1, :, 0:NPIX])
            nc.scalar.dma_start(out=xq_t[1][64:128, :, :], in_=xsp[1, :, WP : NPIX + WP])
            nc.scalar.dma_start(out=wm_t[1][:], in_=wmod[1, :, :])

            for b in range(BPC):
                for t in range(NT):
                    s = _MOD_SLOT.get(t)
                    w_ap = (
                        wm_t[b][:, s * 64 : (s + 1) * 64]
                        if s is not None
                        else wsh_sb[:, t * 64 : (t + 1) * 64]
                    )
                    i, j = _tile_tap(t)
                    xt = xq_t[b] if 78 <= t < 84 else xp_t[b]
                    for cp in range(4):
                        for half in (0, 1):
                            r0 = i + 8 * (2 * cp + half)
                            nc.tensor.matmul(
                                ps_t[b][cp][64 * half : 64 * (half + 1), :],
                                lhsT=w_ap,
                                rhs=xt[:, r0 : r0 + 8, j : j + 64],
                                start=(t == 0),
                                stop=(t == NT - 1),
                                tile_position=(0, 64 * half),
                                skip_group_check=True,
                            )
                for cp in range(4):
                    ot = opool.tile([128, CHUNK], f32, name=f"ot{b}{cp}")
                    nc.vector.tensor_copy(ot[:], ps_t[b][cp][:])
                    # chunk-major y: [2 chunks, 64 ch, 512 px] flat-matches
                    # ot's [128, 512]; one DMA per bank, triggers split
                    # across the two hwdge queues to shorten the tail
                    eng = nc.sync if cp < 2 else nc.scalar
                    eng.dma_start(
                        out=y[b, 2 * cp : 2 * cp + 2, :, :], in_=ot[:, :]
                    )
    nc.compile()
    return nc


def _get_nc():
    global _CACHED_NC
    if _CACHED_NC is None:
        _CACHED_NC = _build_nc()
    return _CACHED_NC


def _host_dyn(x, w1, b1, w2, b2):
    """dwc_proj MLP on host, float64: dyn [B, 64, 9]."""
    pooled = x[:, :PDIM].mean(axis=(2, 3), dtype=np.float64)      # [B, 64]
    z = pooled @ w1.T.astype(np.float64) + b1.astype(np.float64)  # [B, 32]
    h = 0.5 * z * (1.0 + _ERF(z / math.sqrt(2.0)))                # exact gelu
    dyn = h @ w2.T.astype(np.float64) + b2.astype(np.float64)     # [B, 576]
    return dyn.reshape(B, PDIM, SK * SK)


def _host_weights(lk_filter, dyn):
    """Build shared tap-pair weight tiles + per-batch modified central tiles.

    Weight tile t [128, 64]: rows 0-63 = lk[o, c, iA, jA].T (tap A), rows
    64-127 = tap B, zeros for the single. lhsT layout [K=c, M=o].
    """
    lkT = lk_filter.transpose(1, 0, 2, 3).astype(np.float32)  # [c, o, i, j]
    Wt = np.zeros((NT, 128, 64), np.float32)
    for t in range(NT):
        i, jA = _tile_tap(t)
        Wt[t, 0:64, :] = lkT[:, :, i, jA]
        if t < 78:
            Wt[t, 64:128, :] = lkT[:, :, i, jA + 1]
        elif t < 84:
            Wt[t, 64:128, :] = lkT[:, :, i + 1, jA]

    ar = np.arange(64)
    Wmod = np.zeros((B, 6, 128, 64), np.float32)
    for ii, i in enumerate((5, 6, 7)):
        t2, t3 = i * 6 + 2, i * 6 + 3
        u = i - 5
        for b in range(B):
            m2 = Wt[t2].copy()
            m3 = Wt[t3].copy()
            m2[64 + ar, ar] += dyn[b, :, u * 3 + 0].astype(np.float32)  # tap (i,5)
            m3[ar, ar] += dyn[b, :, u * 3 + 1].astype(np.float32)       # tap (i,6)
            m3[64 + ar, ar] += dyn[b, :, u * 3 + 2].astype(np.float32)  # tap (i,7)
            Wmod[b, ii] = m2
            Wmod[b, 3 + ii] = m3

    wsh_np = np.ascontiguousarray(
        Wt.transpose(1, 0, 2).reshape(128, NT * 64)
    ).astype(np.float16)
    wmod_np = np.ascontiguousarray(
        Wmod.transpose(0, 2, 1, 3).reshape(B, 128, 6 * 64)
    ).astype(np.float16)
    return wsh_np, wmod_np


def _make_in_maps(x, lk_filter, w1, b1, w2, b2):
    x = np.asarray(x, dtype=np.float32)
    dyn = _host_dyn(x, np.asarray(w1), np.asarray(b1), np.asarray(w2), np.asarray(b2))
    wsh_np, wmod_np = _host_weights(np.asarray(lk_filter, dtype=np.float32), dyn)

    # flat 77x76-row padded image per channel (see _build_nc)
    xsp_np = np.zeros((B, PDIM, HP + 1, WP), np.float16)
    xsp_np[:, :, PAD : PAD + H, PAD : PAD + W] = x[:, :PDIM]
    xsp_np = xsp_np.reshape(B, PDIM, (HP + 1) * WP)

    in_maps = []
    for k in range(NCORES):
        b0 = k * BPC
        in_maps.append(
            {
                "xsp": np.ascontiguousarray(xsp_np[b0 : b0 + BPC]),
                "wsh": wsh_np,
                "wmod": np.ascontiguousarray(wmod_np[b0 : b0 + BPC]),
            }
        )
    return in_maps


def kernel(x, lk_filter, w1, b1, w2, b2):
    from concourse.bass_utils import run_bass_kernel_spmd

    x = np.asarray(x, dtype=np.float32)
    in_maps = _make_in_maps(x, lk_filter, w1, b1, w2, b2)
    nc = _get_nc()
    res = run_bass_kernel_spmd(nc, in_maps, core_ids=list(range(NCORES)))

    out = np.empty((B, C, H, W), np.float32)
    for k in range(NCORES):
        b0 = k * BPC
        # y is chunk-major [BPC, 8, 64, 512] -> [BPC, 64, 64, 64]
        yk = res.results[k]["y"].reshape(BPC, NCHUNK, PDIM, 8, W)
        out[b0 : b0 + BPC, :PDIM] = yk.transpose(0, 2, 1, 3, 4).reshape(
            BPC, PDIM, H, W
        )
    out[:, PDIM:] = x[:, PDIM:]
    return out


# revision 15
# speedup vs baseline: 1.0295x; 1.0127x over previous
"""Trainium2 Bass kernel for nn_ConvolutionalAttention_3015067042131.

Math (reference.py):
  x [16,128,64,64] f32; x1 = x[:, :64], x2 = x[:, 64:]
  pooled = mean(x1, HW); h = gelu(pooled @ w1.T + b1); dyn = (h @ w2.T + b2) -> [B,64,9]
  x1_dyn = per-(batch,channel) 3x3 depthwise conv of x1 with dyn
  x1_lk  = conv2d(x1, lk_filter[64,64,13,13], SAME)
  out = concat([x1_lk + x1_dyn, x2], ch)

Strategy:
  * The tiny MLP (dyn) is computed on host in float64 (0.0007% of FLOPs).
  * The dynamic depthwise 3x3 is folded into the 13x13 conv weights as
    per-batch diagonal additions on the central 3x3 taps (3x3 tap (u,v)
    == 13x13 tap (u+5, v+5)).
  * Conv as shift-and-matmul: for each kernel tap, out[o, pix] +=
    W_tap[c, o].T @ xpad[c, pix+off]. Taps are paired along K=128:
      - xp layout: partitions 0-63 hold the zero-padded 76x76 image,
        64-127 hold it shifted LEFT one column -> taps (i,2j),(i,2j+1)
        fuse into one matmul. 78 tiles cover columns 0-11.
      - xq layout: partitions 64-127 hold the image shifted UP one row
        -> taps (2i,12),(2i+1,12) fuse. 6 tiles cover column 12 rows
        0-11, plus 1 single tile for tap (12,12).
    85 tiles total for 169 taps (optimal: 84 pairs + 1 single).
  * Loop order: batch outer, tap-tile outer, chunk inner. Each batch's
    full output (4 chunk-pairs x [128,512] f32) stays RESIDENT in PSUM
    (8 banks = exactly 2 images), accumulating across all 85 taps; one
    drain at the end of each batch, pipelined per bank.
  * Chunk pairs run CONCURRENTLY in the two PE column halves via
    tile_position (0,0)/(0,64) writing PSUM partitions 0-63/64-127.
  * fp16 operands (HW fp16 matmul, fp32 PSUM accumulate; end-to-end rel
    err ~3e-4). f32r is broken in this stack; fp32 runs at 1/4 rate;
    fp8 measured 3.8e-2 rel err on this data -> over the 2e-2 gate.
  * Head: the framework preamble is ~7.2us; immediately after it, junk
    matmuls ramp the PE clock (1.2->2.4GHz takes ~3us of sustained PE
    activity) while image DMAs (sync queue) + weight DMAs (scalar
    queue) land and DVE builds the padded layouts. Border memsets run
    during the DMA wait; per-batch layouts build one batch ahead.
  * Sharding: data-parallel over batch, 2 batches per core on 8 cores.
    x2 passthrough is host-side (no device work).
"""
import math

import numpy as np

B, C, H, W = 16, 128, 64, 64
PDIM, SK, LK = 64, 3, 13
PAD = LK // 2  # 6
HP, WP = H + 2 * PAD, W + 2 * PAD  # 76, 76
NCORES = 8
BPC = B // NCORES  # batches per core
NT = 85            # weight tiles (84 tap pairs + 1 single)
NCHUNK = 8         # 512-pixel chunks per image
CHUNK = H * W // NCHUNK  # 512
NWARM = 11         # clock-ramp junk matmuls (N=256 each)

# tile t: t < 78 -> col-pair, A tap (t//6, 2*(t%6)), B = (i, j+1), layout xp
#         78 <= t < 84 -> row-pair, A tap (2*(t-78), 12), B = (i+1, 12), xq
#         t == 84 -> single tap (12, 12), B-half weights zero, layout xp


def _tile_tap(t):
    if t < 78:
        return t // 6, 2 * (t % 6)
    if t < 84:
        return 2 * (t - 78), 12
    return 12, 12


# central 3x3 taps (i,j in 5..7): j=5 -> tile i*6+2 B-half; j=6/7 -> tile
# i*6+3 A/B-half; those 6 tiles are per-batch.
_MOD_TILES = [5 * 6 + 2, 6 * 6 + 2, 7 * 6 + 2, 5 * 6 + 3, 6 * 6 + 3, 7 * 6 + 3]
_MOD_SLOT = {t: s for s, t in enumerate(_MOD_TILES)}

_ERF = np.vectorize(math.erf, otypes=[np.float64])

_CACHED_NC = None


def _build_nc():
    import concourse.mybir as mybir
    import concourse.tile as tile
    from concourse import bacc

    f32 = mybir.dt.float32
    f16 = mybir.dt.float16

    nc = bacc.Bacc(None, target_bir_lowering=False)
    # host-pre-padded image, flat 77x76 rows per channel: rows 0-75 = the
    # zero-padded 76x76 image, row 76 = zeros. The three on-chip layouts
    # are then pure contiguous DMAs at flat offsets 0 / +1 (left-shift
    # wraps col 75 onto the next row's zero pad col) / +76 (up-shift
    # wraps row 75 onto the zero row 76).
    xsp = nc.dram_tensor("xsp", [BPC, PDIM, (HP + 1) * WP], f16, kind="ExternalInput")
    wsh = nc.dram_tensor("wsh", [128, NT * 64], f16, kind="ExternalInput")
    wmod = nc.dram_tensor("wmod", [BPC, 128, 6 * 64], f16, kind="ExternalInput")
    # chunk-major output so each PSUM bank drains in ONE dma
    y = nc.dram_tensor("y", [BPC, NCHUNK, PDIM, CHUNK], f32, kind="ExternalOutput")

    with tile.TileContext(nc) as tc:
        with (
            tc.tile_pool(name="wpool", bufs=1) as wpool,
            tc.tile_pool(name="wmpool", bufs=1) as wmpool,
            tc.tile_pool(name="xppool", bufs=1) as xppool,
            tc.tile_pool(name="xqpool", bufs=1) as xqpool,
            tc.tile_pool(name="opool", bufs=1) as opool,
            tc.tile_pool(name="pspool", bufs=1, space="PSUM") as pspool,
        ):
            # 8 PSUM banks = exactly the per-core output (2 images x 4
            # chunk-pair banks); resident for the whole batch accumulation
            ps_t = [
                [
                    pspool.tile([128, CHUNK], f32, name=f"ps{b}{cp}")
                    for cp in range(4)
                ]
                for b in range(BPC)
            ]
            # PE warmup: junk matmuls on a zeroed scratch tile so the HAM
            # un-throttles (1.2 -> 2.4 GHz) while input DMAs + layout
            # build run. N=256 keeps each one cheap. Targets the LAST bank
            # to be opened for real accumulation (b1 cp3, ~77us away).
            scratch = wpool.tile([128, 512], f16)
            nc.vector.memset(scratch[:], 0.0)
            for wi in range(NWARM):
                nc.tensor.matmul(
                    ps_t[BPC - 1][3][0:64, 0:256],
                    lhsT=scratch[:, 0:64],
                    rhs=scratch[:, 0:256],
                    start=(wi == 0),
                    stop=(wi == NWARM - 1),
                    skip_group_check=True,
                )

            # The DMA queue is determined by the trigger engine (sync -> q1,
            # scalar -> q10, ~170GB/s each), so the batch-0 critical bytes
            # are split across BOTH queues, row-split so region deps let
            # early chunk-pairs start before the full image lands.
            NPIX = HP * WP  # 5776
            RS = 48 * WP    # row-split point (rows 0-47 / 48-75)
            wsh_sb = wpool.tile([128, NT * 64], f16)
            xp_t = [xppool.tile([128, HP, WP], f16, name=f"xp{b}") for b in range(BPC)]
            xq_t = [xqpool.tile([128, HP, WP], f16, name=f"xq{b}") for b in range(BPC)]
            wm_t = [wmpool.tile([128, 6 * 64], f16, name=f"wm{b}") for b in range(BPC)]

            # sync queue: xp0 A-half in 16-row pieces (the first matmuls
            # need only rows 0-16; region deps unblock per piece), then
            # the mid weight chunk
            for r0, r1 in ((0, 16), (16, 32), (32, 48), (48, 64), (64, 76)):
                nc.sync.dma_start(
                    out=xp_t[0][0:64, r0:r1, :], in_=xsp[0, :, r0 * WP : r1 * WP]
                )
            nc.sync.dma_start(
                out=wsh_sb[:, 8 * 64 : 31 * 64], in_=wsh[:, 8 * 64 : 31 * 64]
            )
            # scalar queue: first weight tiles (tiny), xp0 B-half pieces,
            # then the weight tail
            nc.scalar.dma_start(out=wsh_sb[:, 0 : 8 * 64], in_=wsh[:, 0 : 8 * 64])
            for r0, r1 in ((0, 16), (16, 32), (32, 48), (48, 64), (64, 76)):
                nc.scalar.dma_start(
                    out=xp_t[0][64:128, r0:r1, :],
                    in_=xsp[0, :, r0 * WP + 1 : r1 * WP + 1],
                )
            nc.scalar.dma_start(out=wsh_sb[:, 31 * 64 :], in_=wsh[:, 31 * 64 :])
            # non-critical inputs (xq0 needed ~60us in, batch 1 ~80us in);
            # must still be issued before any drain trigger shares a queue
            nc.scalar.dma_start(out=wm_t[0][:], in_=wmod[0, :, :])
            nc.sync.dma_start(out=xq_t[0][0:64, :, :], in_=xsp[0, :, 0:NPIX])
            nc.scalar.dma_start(out=xq_t[0][64:128, :, :], in_=xsp[0, :, WP : NPIX + WP])
            nc.sync.dma_start(out=xp_t[1][0:64, :, :], in_=xsp[1, :, 0:NPIX])
            nc.scalar.dma_start(out=xp_t[1][64:128, :, :], in_=xsp[1, :, 1 : NPIX + 1])
            nc.sync.dma_start(out=xq_t[1][0:64, :, :], in_=xsp[1, :, 0:NPIX])
            nc.scalar.dma_start(out=xq_t[1][64:128, :, :], in_=xsp[1, :, WP : NPIX + WP])
            nc.scalar.dma_start(out=wm_t[1][:], in_=wmod[1, :, :])

            for b in range(BPC):
                for t in range(NT):
                    s = _MOD_SLOT.get(t)
                    w_ap = (
                        wm_t[b][:, s * 64 : (s + 1) * 64]
                        if s is not None
                        else wsh_sb[:, t * 64 : (t + 1) * 64]
                    )
                    i, j = _tile_tap(t)
                    xt = xq_t[b] if 78 <= t < 84 else xp_t[b]
                    for cp in range(4):
                        for half in (0, 1):
                            r0 = i + 8 * (2 * cp + half)
                            nc.tensor.matmul(
                                ps_t[b][cp][64 * half : 64 * (half + 1), :],
                                lhsT=w_ap,
                                rhs=xt[:, r0 : r0 + 8, j : j + 64],
                                start=(t == 0),
                                stop=(t == NT - 1),
                                tile_position=(0, 64 * half),
                                skip_group_check=True,
                            )
                for cp in range(4):
                    ot = opool.tile([128, CHUNK], f32, name=f"ot{b}{cp}")
                    nc.vector.tensor_copy(ot[:], ps_t[b][cp][:])
                    # chunk-major y; each bank's two chunks drain on the two
                    # hwdge queues concurrently to shorten the tail
                    nc.sync.dma_start(out=y[b, 2 * cp, :, :], in_=ot[0:64, :])
                    nc.scalar.dma_start(
                        out=y[b, 2 * cp + 1, :, :], in_=ot[64:128, :]
                    )
    nc.compile()
    return nc


def _get_nc():
    global _CACHED_NC
    if _CACHED_NC is None:
        _CACHED_NC = _build_nc()
    return _CACHED_NC


def _host_dyn(x, w1, b1, w2, b2):
    """dwc_proj MLP on host, float64: dyn [B, 64, 9]."""
    pooled = x[:, :PDIM].mean(axis=(2, 3), dtype=np.float64)      # [B, 64]
    z = pooled @ w1.T.astype(np.float64) + b1.astype(np.float64)  # [B, 32]
    h = 0.5 * z * (1.0 + _ERF(z / math.sqrt(2.0)))                # exact gelu
    dyn = h @ w2.T.astype(np.float64) + b2.astype(np.float64)     # [B, 576]
    return dyn.reshape(B, PDIM, SK * SK)


def _host_weights(lk_filter, dyn):
    """Build shared tap-pair weight tiles + per-batch modified central tiles.

    Weight tile t [128, 64]: rows 0-63 = lk[o, c, iA, jA].T (tap A), rows
    64-127 = tap B, zeros for the single. lhsT layout [K=c, M=o].
    """
    lkT = lk_filter.transpose(1, 0, 2, 3).astype(np.float32)  # [c, o, i, j]
    Wt = np.zeros((NT, 128, 64), np.float32)
    for t in range(NT):
        i, jA = _tile_tap(t)
        Wt[t, 0:64, :] = lkT[:, :, i, jA]
        if t < 78:
            Wt[t, 64:128, :] = lkT[:, :, i, jA + 1]
        elif t < 84:
            Wt[t, 64:128, :] = lkT[:, :, i + 1, jA]

    ar = np.arange(64)
    Wmod = np.zeros((B, 6, 128, 64), np.float32)
    for ii, i in enumerate((5, 6, 7)):
        t2, t3 = i * 6 + 2, i * 6 + 3
        u = i - 5
        for b in range(B):
            m2 = Wt[t2].copy()
            m3 = Wt[t3].copy()
            m2[64 + ar, ar] += dyn[b, :, u * 3 + 0].astype(np.float32)  # tap (i,5)
            m3[ar, ar] += dyn[b, :, u * 3 + 1].astype(np.float32)       # tap (i,6)
            m3[64 + ar, ar] += dyn[b, :, u * 3 + 2].astype(np.float32)  # tap (i,7)
            Wmod[b, ii] = m2
            Wmod[b, 3 + ii] = m3

    wsh_np = np.ascontiguousarray(
        Wt.transpose(1, 0, 2).reshape(128, NT * 64)
    ).astype(np.float16)
    wmod_np = np.ascontiguousarray(
        Wmod.transpose(0, 2, 1, 3).reshape(B, 128, 6 * 64)
    ).astype(np.float16)
    return wsh_np, wmod_np


def _make_in_maps(x, lk_filter, w1, b1, w2, b2):
    x = np.asarray(x, dtype=np.float32)
    dyn = _host_dyn(x, np.asarray(w1), np.asarray(b1), np.asarray(w2), np.asarray(b2))
    wsh_np, wmod_np = _host_weights(np.asarray(lk_filter, dtype=np.float32), dyn)

    # flat 77x76-row padded image per channel (see _build_nc)
    xsp_np = np.zeros((B, PDIM, HP + 1, WP), np.float16)
    xsp_np[:, :, PAD : PAD + H, PAD : PAD + W] = x[:, :PDIM]
    xsp_np = xsp_np.reshape(B, PDIM, (HP + 1) * WP)

    in_maps = []
    for k in range(NCORES):
        b0 = k * BPC
        in_maps.append(
            {
                "xsp": np.ascontiguousarray(xsp_np[b0 : b0 + BPC]),
                "wsh": wsh_np,
                "wmod": np.ascontiguousarray(wmod_np[b0 : b0 + BPC]),
            }
        )
    return in_maps


def kernel(x, lk_filter, w1, b1, w2, b2):
    from concourse.bass_utils import run_bass_kernel_spmd

    x = np.asarray(x, dtype=np.float32)
    in_maps = _make_in_maps(x, lk_filter, w1, b1, w2, b2)
    nc = _get_nc()
    res = run_bass_kernel_spmd(nc, in_maps, core_ids=list(range(NCORES)))

    out = np.empty((B, C, H, W), np.float32)
    for k in range(NCORES):
        b0 = k * BPC
        # y is chunk-major [BPC, 8, 64, 512] -> [BPC, 64, 64, 64]
        yk = res.results[k]["y"].reshape(BPC, NCHUNK, PDIM, 8, W)
        out[b0 : b0 + BPC, :PDIM] = yk.transpose(0, 2, 1, 3, 4).reshape(
            BPC, PDIM, H, W
        )
    out[:, PDIM:] = x[:, PDIM:]
    return out
